# revision 1
# baseline (speedup 1.0000x reference)
"""SkipGram negative-sampling loss kernel for 8 Trainium2 NeuronCores.

Strategy: data-parallel over walks (batch). The 1M x 128 embedding table is
replicated to every core's HBM; each core handles B/8 = 128 walks:
  - per-column indirect-DMA gathers (128 rows / instruction) of the walk and
    neg embeddings into SBUF, laid out [walk -> partition, position*D -> free]
  - cast to bf16 (2x DVE mode), dot products via elementwise mult +
    pairwise-halving add + segmented reduce
  - softplus via ScalarE Exp then Ln(x+1) with per-partition accumulate
    (accum_out accumulates on HW; zeroed once)
  - each core returns [128, 1] partial sums; host sums and divides.
"""

import sys
import types

import numpy as np

try:  # missing in some containers; shim so trace=True degrades gracefully
    from antenv.axon_hooks import get_axon_ntff_profile_hook  # noqa: F401
except Exception:
    _m = types.ModuleType("antenv.axon_hooks")
    _m.get_axon_ntff_profile_hook = lambda: None
    sys.modules["antenv.axon_hooks"] = _m

import concourse.bass as bass
import concourse.bacc as bacc
import concourse.tile as tile
import concourse.mybir as mybir
from concourse.bass_utils import run_bass_kernel_spmd

F32 = mybir.dt.float32
BF16 = mybir.dt.bfloat16
I32 = mybir.dt.int32

N_CORES = 8


def build_kernel(n_walks, L, A, NEG, D, n_nodes, n_cores=N_CORES, reps=1, debug=False):
    """Build the SPMD Bass module (same NEFF on every core).

    reps > 1 repeats the whole workload (for slope timing); rep r writes its
    partials to out[:, r].
    """
    W1 = L - A  # window_size - 1 = number of pos offsets (4)
    H = D // 2
    nc = bacc.Bacc(
        "TRN2",
        target_bir_lowering=False,
        debug=False,
        num_devices=n_cores,
    )
    walk_idx = nc.dram_tensor("walk_idx", [n_walks, L], I32, kind="ExternalInput")
    neg_idx = nc.dram_tensor("neg_idx", [n_walks, NEG * A], I32, kind="ExternalInput")
    embed = nc.dram_tensor("embed", [n_nodes, D], F32, kind="ExternalInput")
    out = nc.dram_tensor("out", [n_walks, reps], F32, kind="ExternalOutput")
    NPdbg = (L - A) + NEG
    if debug:
        ew_o = nc.dram_tensor("ew_o", [n_walks, L * D], F32, kind="ExternalOutput")
        en_o = nc.dram_tensor("en_o", [n_walks, A * D], F32, kind="ExternalOutput")
        logit_o = nc.dram_tensor(
            "logit_o", [n_walks, NPdbg * A], F32, kind="ExternalOutput"
        )
        sp_o = nc.dram_tensor("sp_o", [n_walks, NPdbg * A], F32, kind="ExternalOutput")

    with tile.TileContext(nc) as tc:
        with (
            tc.tile_pool(name="idx", bufs=1) as idxp,
            tc.tile_pool(name="g32", bufs=2) as g32p,   # f32 gather landing
            tc.tile_pool(name="ew16", bufs=1) as ew16p,
            tc.tile_pool(name="en16", bufs=2) as en16p,
            tc.tile_pool(name="prod", bufs=2) as prodp,
            tc.tile_pool(name="half", bufs=1) as halfp,
            tc.tile_pool(name="small", bufs=2) as smallp,
            tc.tile_pool(name="accp", bufs=1) as accp,
        ):
            wi = idxp.tile([n_walks, L], I32)
            nc.sync.dma_start(out=wi[:], in_=walk_idx[:])
            ni = idxp.tile([n_walks, NEG * A], I32)
            nc.sync.dma_start(out=ni[:], in_=neg_idx[:])

            NP = W1 + NEG  # planes per rep
            acc = accp.tile([n_walks, reps * NP], F32)

            def gather_cast(idx_ap, ncols, out16_ap):
                """column-wise gather of ncols rows/partition + cast to bf16"""
                t32 = g32p.tile([n_walks, 80 * D], F32, tag="g32")
                for k in range(ncols):
                    nc.gpsimd.indirect_dma_start(
                        out=t32[:, k * D : (k + 1) * D],
                        out_offset=None,
                        in_=embed[:],
                        in_offset=bass.IndirectOffsetOnAxis(
                            ap=idx_ap[:, k : k + 1], axis=0
                        ),
                    )
                nc.vector.tensor_copy(out16_ap, t32[:, : ncols * D])

            for r in range(reps):
                ew16 = ew16p.tile([n_walks, L * D], BF16)
                gather_cast(wi, L, ew16[:])
                if debug and r == 0:
                    dbgf = g32p.tile([n_walks, 80 * D], F32, tag="g32")
                    nc.vector.tensor_copy(dbgf[:, : L * D], ew16[:])
                    nc.sync.dma_start(out=ew_o[:], in_=dbgf[:, : L * D])
                plane_ctr = [0]

                def dot_softplus_accum(other16_ap, sp_scale):
                    prod = prodp.tile([n_walks, A * D], BF16)
                    nc.vector.tensor_mul(prod[:], ew16[:, 0 : A * D], other16_ap)
                    p3 = prod[:].rearrange("p (a d) -> p a d", d=D)
                    half = halfp.tile([n_walks, A * H], BF16)
                    h3 = half[:].rearrange("p (a d) -> p a d", d=H)
                    nc.vector.tensor_add(h3, p3[:, :, 0:H], p3[:, :, H:D])
                    logit = smallp.tile([n_walks, A], F32)
                    nc.vector.tensor_reduce(
                        logit[:],
                        h3,
                        axis=mybir.AxisListType.X,
                        op=mybir.AluOpType.add,
                    )
                    # stable softplus(s*x) = max(s*x,0) + ln(1 + exp(-|x|));
                    # naive ln(exp(x)+1) breaks on HW act tables for |x|>~50
                    ab = smallp.tile([n_walks, A], F32)
                    nc.scalar.activation(
                        ab[:], logit[:], mybir.ActivationFunctionType.Abs
                    )
                    e = smallp.tile([n_walks, A], F32)
                    nc.scalar.activation(
                        e[:], ab[:], mybir.ActivationFunctionType.Exp, scale=-1.0
                    )
                    ln1 = smallp.tile([n_walks, A], F32)
                    nc.scalar.activation(
                        ln1[:], e[:], mybir.ActivationFunctionType.Ln, bias=1.0
                    )
                    rl = smallp.tile([n_walks, A], F32)
                    nc.vector.tensor_scalar(
                        rl[:],
                        logit[:],
                        sp_scale,
                        0.0,
                        mybir.AluOpType.mult,
                        mybir.AluOpType.max,
                    )
                    sp = smallp.tile([n_walks, A], F32)
                    col = r * NP + plane_ctr[0]
                    plane_ctr[0] += 1
                    nc.vector.tensor_add(sp[:], ln1[:], rl[:])
                    nc.vector.tensor_reduce(
                        acc[:, col : col + 1],
                        sp[:],
                        axis=mybir.AxisListType.X,
                        op=mybir.AluOpType.add,
                    )
                    if debug and r == 0:
                        pidx = plane_ctr[0] - 1
                        nc.sync.dma_start(
                            out=logit_o[:, pidx * A : (pidx + 1) * A], in_=logit[:]
                        )
                        nc.sync.dma_start(
                            out=sp_o[:, pidx * A : (pidx + 1) * A], in_=sp[:]
                        )

                for i in range(1, W1 + 1):
                    dot_softplus_accum(ew16[:, i * D : (i + A) * D], -1.0)

                for j in range(NEG):
                    en16 = en16p.tile([n_walks, A * D], BF16)
                    gather_cast(ni[:, j * A : (j + 1) * A], A, en16[:])
                    if debug and r == 0 and j == 0:
                        dbgf2 = g32p.tile([n_walks, 80 * D], F32, tag="g32")
                        nc.vector.tensor_copy(dbgf2[:, : A * D], en16[:])
                        nc.sync.dma_start(out=en_o[:], in_=dbgf2[:, : A * D])
                    dot_softplus_accum(en16[:], 1.0)

            # per-rep sum of the NP plane columns -> out[:, r]
            osum = accp.tile([n_walks, reps], F32)
            nc.vector.tensor_reduce(
                osum[:],
                acc[:].rearrange("p (r n) -> p r n", n=NP),
                axis=mybir.AxisListType.X,
                op=mybir.AluOpType.add,
            )
            nc.sync.dma_start(out=out[:], in_=osum[:])

    nc.compile()
    return nc


_NC_CACHE = {}


def _get_nc(key):
    if key not in _NC_CACHE:
        _NC_CACHE[key] = build_kernel(*key)
    return _NC_CACHE[key]


def make_in_maps(walk, neg, embed, n_cores=N_CORES):
    B, L = walk.shape
    A, NEG = neg.shape[1], neg.shape[2]
    nw = B // n_cores
    embed_f = np.ascontiguousarray(embed.astype(np.float32, copy=False))
    in_maps = []
    for c in range(n_cores):
        sl = slice(c * nw, (c + 1) * nw)
        wslice = np.ascontiguousarray(walk[sl].astype(np.int32, copy=False))
        # neg [nw, A, NEG] -> plane-major [nw, NEG*A]
        nslice = np.ascontiguousarray(
            neg[sl].astype(np.int32, copy=False).transpose(0, 2, 1).reshape(nw, NEG * A)
        )
        in_maps.append({"walk_idx": wslice, "neg_idx": nslice, "embed": embed_f})
    return in_maps


def kernel(walk, neg, embed, _trace=False):
    walk = np.asarray(walk)
    neg = np.asarray(neg)
    embed = np.asarray(embed)
    B, L = walk.shape
    A, NEG = neg.shape[1], neg.shape[2]
    n_nodes, D = embed.shape
    nw = B // N_CORES

    nc = _get_nc((nw, L, A, NEG, D, n_nodes, N_CORES))
    in_maps = make_in_maps(walk, neg, embed)
    res = run_bass_kernel_spmd(
        nc, in_maps, core_ids=list(range(N_CORES)), trace=_trace
    )
    total = 2 * B * A * NEG
    s = sum(r["out"][:, 0].astype(np.float64).sum() for r in res.results)
    loss = np.float32(s / total)
    if _trace:
        return loss, res
    return loss



# revision 2
# speedup vs baseline: 3.6260x; 3.6260x over previous
"""SkipGram negative-sampling loss kernel for 8 Trainium2 NeuronCores.

Strategy: data-parallel over walks (batch). The 1M x 128 embedding table is
replicated to every core's HBM; each core handles B/8 = 128 walks (one walk
per SBUF partition):
  - 5 large indirect-DMA gathers (walk: 10240 rows; 4 neg planes: 9728 rows
    each) with f32->bf16 cast during DMA. Large gathers amortize the ~1us
    SWDGE per-instruction overhead that dominated the per-column version.
  - dot products via bf16 tensor_tensor multiply (2x DVE mode) + halving-add
    tree (2x) + short 1x reduce
  - softplus batched over all 8 planes: one Abs/Exp/Ln pass each (3 ACT
    table loads total)
  - each core returns [128, 1] partial sums; host sums and divides.
"""

import sys
import types

import numpy as np

try:  # missing in some containers; shim so trace=True degrades gracefully
    from antenv.axon_hooks import get_axon_ntff_profile_hook  # noqa: F401
except Exception:
    _m = types.ModuleType("antenv.axon_hooks")
    _m.get_axon_ntff_profile_hook = lambda: None
    sys.modules["antenv.axon_hooks"] = _m

import concourse.bass as bass
import concourse.bacc as bacc
import concourse.tile as tile
import concourse.mybir as mybir
from concourse.bass_utils import run_bass_kernel_spmd

F32 = mybir.dt.float32
BF16 = mybir.dt.bfloat16
I32 = mybir.dt.int32

N_CORES = 8


def build_kernel(n_walks, L, A, NEG, D, n_nodes, n_cores=N_CORES):
    """Build the SPMD Bass module (same NEFF on every core)."""
    W1 = L - A  # window_size - 1 = number of pos offsets (4)
    NP = W1 + NEG  # 8 logit planes
    nc = bacc.Bacc(
        "TRN2",
        target_bir_lowering=False,
        debug=False,
        num_devices=n_cores,
    )
    walk_idx = nc.dram_tensor("walk_idx", [n_walks, L], I32, kind="ExternalInput")
    neg_idx = nc.dram_tensor("neg_idx", [n_walks, NEG * A], I32, kind="ExternalInput")
    embed = nc.dram_tensor("embed", [n_nodes, D], F32, kind="ExternalInput")
    out = nc.dram_tensor("out", [n_walks, 1], F32, kind="ExternalOutput")

    with tile.TileContext(nc) as tc:
        with (
            tc.tile_pool(name="idx", bufs=1) as idxp,
            tc.tile_pool(name="ew", bufs=1) as ewp,
            tc.tile_pool(name="en", bufs=1) as enp,
            tc.tile_pool(name="prod", bufs=2) as prodp,
            tc.tile_pool(name="tree", bufs=2) as treep,
            tc.tile_pool(name="small", bufs=1) as smallp,
        ):
            wi = idxp.tile([n_walks, L], I32)
            nc.sync.dma_start(out=wi[:], in_=walk_idx[:])
            ni = idxp.tile([n_walks, NEG * A], I32)
            nc.sync.dma_start(out=ni[:], in_=neg_idx[:])

            # one big gather for the whole walk (L rows per partition)
            ew16 = ewp.tile([n_walks, L * D], BF16)
            nc.gpsimd.indirect_dma_start(
                out=ew16[:],
                out_offset=None,
                in_=embed[:],
                in_offset=bass.IndirectOffsetOnAxis(ap=wi[:, 0:L], axis=0),
            )
            # one gather per neg plane (A rows per partition each)
            en16 = []
            for j in range(NEG):
                t = enp.tile([n_walks, A * D], BF16)
                nc.gpsimd.indirect_dma_start(
                    out=t[:],
                    out_offset=None,
                    in_=embed[:],
                    in_offset=bass.IndirectOffsetOnAxis(
                        ap=ni[:, j * A : (j + 1) * A], axis=0
                    ),
                )
                en16.append(t)

            logits = smallp.tile([n_walks, NP * A], F32)

            def dot_plane(other16_ap, col):
                """logits[:, col*A:(col+1)*A] = per-anchor dot(anc, other)."""
                prod = prodp.tile([n_walks, A * D], BF16)
                nc.vector.tensor_mul(prod[:], ew16[:, 0 : A * D], other16_ap)
                # halving-add tree in bf16 (2x DVE), then short 1x reduce
                cur = prod[:].rearrange("p (a d) -> p a d", d=D)
                w = D
                while w > 8:
                    h = w // 2
                    nt = treep.tile([n_walks, A * h], BF16, tag=f"t{h}")
                    n3 = nt[:].rearrange("p (a d) -> p a d", d=h)
                    nc.vector.tensor_add(n3, cur[:, :, 0:h], cur[:, :, h:w])
                    cur = n3
                    w = h
                nc.vector.tensor_reduce(
                    logits[:, col * A : (col + 1) * A],
                    cur,
                    axis=mybir.AxisListType.X,
                    op=mybir.AluOpType.add,
                )

            for i in range(1, W1 + 1):
                dot_plane(ew16[:, i * D : (i + A) * D], i - 1)
            for j in range(NEG):
                dot_plane(en16[j][:], W1 + j)

            # batched stable softplus over all NP*A logits:
            #   softplus(s*x) = max(s*x, 0) + ln(1 + exp(-|x|))
            # (s = -1 for pos planes, +1 for neg planes)
            ab = smallp.tile([n_walks, NP * A], F32)
            nc.scalar.activation(ab[:], logits[:], mybir.ActivationFunctionType.Abs)
            e = smallp.tile([n_walks, NP * A], F32)
            nc.scalar.activation(
                e[:], ab[:], mybir.ActivationFunctionType.Exp, scale=-1.0
            )
            ln1 = smallp.tile([n_walks, NP * A], F32)
            nc.scalar.activation(
                ln1[:], e[:], mybir.ActivationFunctionType.Ln, bias=1.0
            )
            rl = smallp.tile([n_walks, NP * A], F32)
            nc.vector.tensor_scalar(
                rl[:, 0 : W1 * A],
                logits[:, 0 : W1 * A],
                -1.0,
                0.0,
                mybir.AluOpType.mult,
                mybir.AluOpType.max,
            )
            nc.vector.tensor_scalar(
                rl[:, W1 * A : NP * A],
                logits[:, W1 * A : NP * A],
                1.0,
                0.0,
                mybir.AluOpType.mult,
                mybir.AluOpType.max,
            )
            sp = smallp.tile([n_walks, NP * A], F32)
            nc.vector.tensor_add(sp[:], ln1[:], rl[:])
            osum = smallp.tile([n_walks, 1], F32)
            nc.vector.tensor_reduce(
                osum[:],
                sp[:],
                axis=mybir.AxisListType.X,
                op=mybir.AluOpType.add,
            )
            nc.sync.dma_start(out=out[:], in_=osum[:])

    nc.compile()
    return nc


_NC_CACHE = {}


def _get_nc(key):
    if key not in _NC_CACHE:
        _NC_CACHE[key] = build_kernel(*key)
    return _NC_CACHE[key]


def make_in_maps(walk, neg, embed, n_cores=N_CORES):
    B, L = walk.shape
    A, NEG = neg.shape[1], neg.shape[2]
    nw = B // n_cores
    embed_f = np.ascontiguousarray(embed.astype(np.float32, copy=False))
    in_maps = []
    for c in range(n_cores):
        sl = slice(c * nw, (c + 1) * nw)
        wslice = np.ascontiguousarray(walk[sl].astype(np.int32, copy=False))
        # neg [nw, A, NEG] -> plane-major [nw, NEG*A]
        nslice = np.ascontiguousarray(
            neg[sl].astype(np.int32, copy=False).transpose(0, 2, 1).reshape(nw, NEG * A)
        )
        in_maps.append({"walk_idx": wslice, "neg_idx": nslice, "embed": embed_f})
    return in_maps


def kernel(walk, neg, embed, _trace=False):
    walk = np.asarray(walk)
    neg = np.asarray(neg)
    embed = np.asarray(embed)
    B, L = walk.shape
    A, NEG = neg.shape[1], neg.shape[2]
    n_nodes, D = embed.shape

    nc = _get_nc((B // N_CORES, L, A, NEG, D, n_nodes, N_CORES))
    in_maps = make_in_maps(walk, neg, embed)
    res = run_bass_kernel_spmd(
        nc, in_maps, core_ids=list(range(N_CORES)), trace=_trace
    )
    total = 2 * B * A * NEG
    s = sum(r["out"].astype(np.float64).sum() for r in res.results)
    loss = np.float32(s / total)
    if _trace:
        return loss, res
    return loss


# revision 3
# speedup vs baseline: 4.9042x; 1.3525x over previous
"""SkipGram negative-sampling loss kernel for 8 Trainium2 NeuronCores.

Strategy: data-parallel over walks (batch). The 1M x 128 embedding table is
replicated to every core's HBM; each core handles B/8 = 128 walks (one walk
per SBUF partition):
  - 5 large indirect-DMA gathers (walk: 10240 rows; 4 neg planes: 9728 rows
    each) with f32->bf16 cast during DMA. Large gathers amortize the ~1us
    SWDGE per-instruction overhead that dominated the per-column version.
  - dot products via bf16 tensor_tensor multiply (2x DVE mode) + halving-add
    tree (2x) + short 1x reduce
  - softplus batched over all 8 planes: one Abs/Exp/Ln pass each (3 ACT
    table loads total)
  - each core returns [128, 1] partial sums; host sums and divides.
"""

import sys
import types

import numpy as np

try:  # missing in some containers; shim so trace=True degrades gracefully
    from antenv.axon_hooks import get_axon_ntff_profile_hook  # noqa: F401
except Exception:
    _m = types.ModuleType("antenv.axon_hooks")
    _m.get_axon_ntff_profile_hook = lambda: None
    sys.modules["antenv.axon_hooks"] = _m

import concourse.bass as bass
import concourse.bacc as bacc
import concourse.tile as tile
import concourse.mybir as mybir
from concourse.bass_utils import run_bass_kernel_spmd

F32 = mybir.dt.float32
BF16 = mybir.dt.bfloat16
I32 = mybir.dt.int32

N_CORES = 8


def build_kernel(n_walks, L, A, NEG, D, n_nodes, n_cores=N_CORES):
    """Build the SPMD Bass module (same NEFF on every core)."""
    W1 = L - A  # window_size - 1 = number of pos offsets (4)
    NP = W1 + NEG  # 8 logit planes
    nc = bacc.Bacc(
        "TRN2",
        target_bir_lowering=False,
        debug=False,
        num_devices=n_cores,
    )
    walk_idx = nc.dram_tensor("walk_idx", [n_walks, L], I32, kind="ExternalInput")
    neg_idx = nc.dram_tensor("neg_idx", [n_walks, NEG * A], I32, kind="ExternalInput")
    embed = nc.dram_tensor("embed", [n_nodes, D], F32, kind="ExternalInput")
    out = nc.dram_tensor("out", [n_walks, 1], F32, kind="ExternalOutput")

    with tile.TileContext(nc) as tc:
        with (
            tc.tile_pool(name="idx", bufs=1) as idxp,
            tc.tile_pool(name="ew", bufs=1) as ewp,
            tc.tile_pool(name="en", bufs=4) as enp,
            tc.tile_pool(name="prod", bufs=2) as prodp,
            tc.tile_pool(name="tree", bufs=2) as treep,
            tc.tile_pool(name="small", bufs=1) as smallp,
        ):
            wi = idxp.tile([n_walks, L], I32)
            nc.sync.dma_start(out=wi[:], in_=walk_idx[:])
            ni = idxp.tile([n_walks, NEG * A], I32)
            nc.sync.dma_start(out=ni[:], in_=neg_idx[:])

            # one big gather for the whole walk (L rows per partition)
            ew16 = ewp.tile([n_walks, L * D], BF16)
            nc.gpsimd.indirect_dma_start(
                out=ew16[:],
                out_offset=None,
                in_=embed[:],
                in_offset=bass.IndirectOffsetOnAxis(ap=wi[:, 0:L], axis=0),
            )
            # one gather per neg plane (A rows per partition each)
            en16 = []
            for j in range(NEG):
                t = enp.tile([n_walks, A * D], BF16)
                nc.gpsimd.indirect_dma_start(
                    out=t[:],
                    out_offset=None,
                    in_=embed[:],
                    in_offset=bass.IndirectOffsetOnAxis(
                        ap=ni[:, j * A : (j + 1) * A], axis=0
                    ),
                )
                en16.append(t)

            logits = smallp.tile([n_walks, NP * A], F32)

            def dot_plane(other16_ap, col):
                """logits[:, col*A:(col+1)*A] = per-anchor dot(anc, other)."""
                prod = prodp.tile([n_walks, A * D], BF16)
                nc.vector.tensor_mul(prod[:], ew16[:, 0 : A * D], other16_ap)
                # halving-add tree in bf16 (2x DVE), then short 1x reduce
                cur = prod[:].rearrange("p (a d) -> p a d", d=D)
                w = D
                while w > 8:
                    h = w // 2
                    nt = treep.tile([n_walks, A * h], BF16, tag=f"t{h}")
                    n3 = nt[:].rearrange("p (a d) -> p a d", d=h)
                    nc.vector.tensor_add(n3, cur[:, :, 0:h], cur[:, :, h:w])
                    cur = n3
                    w = h
                nc.vector.tensor_reduce(
                    logits[:, col * A : (col + 1) * A],
                    cur,
                    axis=mybir.AxisListType.X,
                    op=mybir.AluOpType.add,
                )

            for i in range(1, W1 + 1):
                dot_plane(ew16[:, i * D : (i + A) * D], i - 1)
            for j in range(NEG):
                dot_plane(en16[j][:], W1 + j)

            # batched stable softplus over all NP*A logits:
            #   softplus(s*x) = max(s*x, 0) + ln(1 + exp(-|x|))
            # (s = -1 for pos planes, +1 for neg planes)
            ab = smallp.tile([n_walks, NP * A], F32)
            nc.scalar.activation(ab[:], logits[:], mybir.ActivationFunctionType.Abs)
            e = smallp.tile([n_walks, NP * A], F32)
            nc.scalar.activation(
                e[:], ab[:], mybir.ActivationFunctionType.Exp, scale=-1.0
            )
            ln1 = smallp.tile([n_walks, NP * A], F32)
            nc.scalar.activation(
                ln1[:], e[:], mybir.ActivationFunctionType.Ln, bias=1.0
            )
            rl = smallp.tile([n_walks, NP * A], F32)
            nc.vector.tensor_scalar(
                rl[:, 0 : W1 * A],
                logits[:, 0 : W1 * A],
                -1.0,
                0.0,
                mybir.AluOpType.mult,
                mybir.AluOpType.max,
            )
            nc.vector.tensor_scalar(
                rl[:, W1 * A : NP * A],
                logits[:, W1 * A : NP * A],
                1.0,
                0.0,
                mybir.AluOpType.mult,
                mybir.AluOpType.max,
            )
            sp = smallp.tile([n_walks, NP * A], F32)
            nc.vector.tensor_add(sp[:], ln1[:], rl[:])
            osum = smallp.tile([n_walks, 1], F32)
            nc.vector.tensor_reduce(
                osum[:],
                sp[:],
                axis=mybir.AxisListType.X,
                op=mybir.AluOpType.add,
            )
            nc.sync.dma_start(out=out[:], in_=osum[:])

    nc.compile()
    return nc


_NC_CACHE = {}


def _get_nc(key):
    if key not in _NC_CACHE:
        _NC_CACHE[key] = build_kernel(*key)
    return _NC_CACHE[key]


def make_in_maps(walk, neg, embed, n_cores=N_CORES):
    B, L = walk.shape
    A, NEG = neg.shape[1], neg.shape[2]
    nw = B // n_cores
    embed_f = np.ascontiguousarray(embed.astype(np.float32, copy=False))
    in_maps = []
    for c in range(n_cores):
        sl = slice(c * nw, (c + 1) * nw)
        wslice = np.ascontiguousarray(walk[sl].astype(np.int32, copy=False))
        # neg [nw, A, NEG] -> plane-major [nw, NEG*A]
        nslice = np.ascontiguousarray(
            neg[sl].astype(np.int32, copy=False).transpose(0, 2, 1).reshape(nw, NEG * A)
        )
        in_maps.append({"walk_idx": wslice, "neg_idx": nslice, "embed": embed_f})
    return in_maps


def kernel(walk, neg, embed, _trace=False):
    walk = np.asarray(walk)
    neg = np.asarray(neg)
    embed = np.asarray(embed)
    B, L = walk.shape
    A, NEG = neg.shape[1], neg.shape[2]
    n_nodes, D = embed.shape

    nc = _get_nc((B // N_CORES, L, A, NEG, D, n_nodes, N_CORES))
    in_maps = make_in_maps(walk, neg, embed)
    res = run_bass_kernel_spmd(
        nc, in_maps, core_ids=list(range(N_CORES)), trace=_trace
    )
    total = 2 * B * A * NEG
    s = sum(r["out"].astype(np.float64).sum() for r in res.results)
    loss = np.float32(s / total)
    if _trace:
        return loss, res
    return loss


# revision 16
# speedup vs baseline: 5.0955x; 1.0390x over previous
"""SkipGram negative-sampling loss kernel for 8 Trainium2 NeuronCores.

Strategy: data-parallel over walks (batch). The 1M x 128 embedding table is
replicated to every core's HBM; each core handles B/8 = 128 walks (one walk
per SBUF partition):
  - 6 large indirect-DMA gathers (walk split in two for an earlier compute
    start; one per neg plane) with f32->bf16 cast during DMA. Large gathers
    amortize the ~1us SWDGE per-instruction overhead.
  - dot products in 16 half-plane chunks (38 anchors each): bf16
    tensor_tensor multiply (2x DVE) + halving-add tree; the first tree level
    of the first POOL_N chunks runs on the otherwise-idle GpSimd engine.
  - softplus via the native ACT Softplus table (preloaded at t=0), one call
    per chunk with accum_out -> per-chunk partial sums; tiny final reduce.
  - each core returns [128, 1] partial sums; host sums and divides.
"""

import sys
import types

import numpy as np

try:  # missing in some containers; shim so trace=True degrades gracefully
    from antenv.axon_hooks import get_axon_ntff_profile_hook  # noqa: F401
except Exception:
    _m = types.ModuleType("antenv.axon_hooks")
    _m.get_axon_ntff_profile_hook = lambda: None
    sys.modules["antenv.axon_hooks"] = _m

import concourse.bass as bass
import concourse.bacc as bacc
import concourse.tile as tile
import concourse.mybir as mybir
from concourse.bass_utils import run_bass_kernel_spmd

F32 = mybir.dt.float32
BF16 = mybir.dt.bfloat16
I32 = mybir.dt.int32

N_CORES = 8
POOL_N = 0  # number of half-plane chunks whose tree level 1 runs on GpSimd
WALK_SPLIT = 42  # walk gather split column (first part covers half-0 pos work)


def build_kernel(n_walks, L, A, NEG, D, n_nodes, n_cores=N_CORES):
    """Build the SPMD Bass module (same NEFF on every core)."""
    W1 = L - A  # window_size - 1 = number of pos offsets (4)
    H = A // 2  # anchors per half-plane chunk (38)
    NCH = 2 * (W1 + NEG)  # 16 chunks
    nc = bacc.Bacc(
        "TRN2",
        target_bir_lowering=False,
        debug=False,
        num_devices=n_cores,
    )
    walk_idx = nc.dram_tensor("walk_idx", [n_walks, L], I32, kind="ExternalInput")
    neg_idx = nc.dram_tensor("neg_idx", [n_walks, NEG * A], I32, kind="ExternalInput")
    embed = nc.dram_tensor("embed", [n_nodes, D], F32, kind="ExternalInput")
    out = nc.dram_tensor("out", [n_walks, 1], F32, kind="ExternalOutput")

    with tile.TileContext(nc) as tc:
        with (
            tc.tile_pool(name="idx", bufs=1) as idxp,
            tc.tile_pool(name="ew", bufs=1) as ewp,
            tc.tile_pool(name="en", bufs=4) as enp,
            tc.tile_pool(name="prod", bufs=6) as prodp,
            tc.tile_pool(name="t1", bufs=3) as t1p,
            tc.tile_pool(name="t2", bufs=2) as t2p,
            tc.tile_pool(name="sp", bufs=1) as spp,
            tc.tile_pool(name="small", bufs=1) as smallp,
        ):
            wi = idxp.tile([n_walks, L], I32)
            nc.sync.dma_start(out=wi[:], in_=walk_idx[:])
            ni = idxp.tile([n_walks, NEG * A], I32)
            nc.sync.dma_start(out=ni[:], in_=neg_idx[:])

            # walk gather, split so early pos chunks can start sooner
            ew16 = ewp.tile([n_walks, L * D], BF16)
            nc.gpsimd.indirect_dma_start(
                out=ew16[:, 0 : WALK_SPLIT * D],
                out_offset=None,
                in_=embed[:],
                in_offset=bass.IndirectOffsetOnAxis(ap=wi[:, 0:WALK_SPLIT], axis=0),
            )
            nc.gpsimd.indirect_dma_start(
                out=ew16[:, WALK_SPLIT * D :],
                out_offset=None,
                in_=embed[:],
                in_offset=bass.IndirectOffsetOnAxis(ap=wi[:, WALK_SPLIT:L], axis=0),
            )
            # one gather per neg plane
            en16 = []
            for j in range(NEG):
                t = enp.tile([n_walks, A * D], BF16)
                nc.gpsimd.indirect_dma_start(
                    out=t[:],
                    out_offset=None,
                    in_=embed[:],
                    in_offset=bass.IndirectOffsetOnAxis(
                        ap=ni[:, j * A : (j + 1) * A], axis=0
                    ),
                )
                en16.append(t)

            logits = smallp.tile([n_walks, NCH * H], F32)

            # chunk list: (anchor_col, other_ap, sign). Order: pos half-0
            # (needs only walk part 1), pos half-1, then neg planes by half.
            chunks = []
            for i in range(1, W1 + 1):
                chunks.append((0, ew16[:, i * D : (i + H) * D], -1.0))
            for i in range(1, W1 + 1):
                chunks.append(
                    (H, ew16[:, (i + H) * D : (i + H + H) * D], -1.0)
                )
            for j in range(NEG):
                for h in range(2):
                    chunks.append(
                        (h * H, en16[j][:, h * H * D : (h + 1) * H * D], 1.0)
                    )

            def mult(k):
                a0, other, _ = chunks[k]
                prod = prodp.tile([n_walks, H * D], BF16)
                nc.vector.tensor_mul(
                    prod[:], ew16[:, a0 * D : (a0 + H) * D], other
                )
                return prod

            def tail(k, prod):
                cur = prod[:].rearrange("p (a d) -> p a d", d=D)
                w = D
                eng = nc.gpsimd if k < POOL_N else nc.vector
                while w > 8:
                    h2 = w // 2
                    pool = t1p if w == D else t2p
                    nt = pool.tile([n_walks, H * h2], BF16, tag=f"t{h2}")
                    n3 = nt[:].rearrange("p (a d) -> p a d", d=h2)
                    eng.tensor_add(n3, cur[:, :, 0:h2], cur[:, :, h2:w])
                    eng = nc.vector  # only level 1 may run on GpSimd
                    cur = n3
                    w = h2
                nc.vector.tensor_reduce(
                    logits[:, k * H : (k + 1) * H],
                    cur,
                    axis=mybir.AxisListType.X,
                    op=mybir.AluOpType.add,
                )

            # interleave: tail(k) is emitted two multiplies later so the
            # GpSimd level-1 of chunk k overlaps DVE multiplies k+1, k+2
            prods = {}
            LAG = 2
            for k in range(NCH):
                prods[k] = mult(k)
                if k - LAG >= 0:
                    tail(k - LAG, prods.pop(k - LAG))
            for k in range(NCH - LAG, NCH):
                tail(k, prods.pop(k))

            # batched stable softplus over all NCH*H logits:
            #   softplus(s*x) = max(s*x, 0) + ln(1 + exp(-|x|))
            # (s = -1 for pos chunks [0, NCH/2), +1 for neg chunks)
            NL = NCH * H
            PL = (NCH // 2) * H  # pos block size
            rl = spp.tile([n_walks, NL], F32)
            nc.vector.tensor_scalar(
                rl[:, 0:PL], logits[:, 0:PL], -1.0, 0.0,
                mybir.AluOpType.mult, mybir.AluOpType.max,
            )
            nc.vector.tensor_scalar(
                rl[:, PL:NL], logits[:, PL:NL], 1.0, 0.0,
                mybir.AluOpType.mult, mybir.AluOpType.max,
            )
            ab = spp.tile([n_walks, NL], F32)
            nc.scalar.activation(ab[:], logits[:], mybir.ActivationFunctionType.Abs)
            e = spp.tile([n_walks, NL], F32)
            nc.scalar.activation(
                e[:], ab[:], mybir.ActivationFunctionType.Exp, scale=-1.0
            )
            ln1 = spp.tile([n_walks, NL], F32)
            nc.scalar.activation(
                ln1[:], e[:], mybir.ActivationFunctionType.Ln, bias=1.0
            )
            sp = spp.tile([n_walks, NL], F32)
            nc.vector.tensor_add(sp[:], ln1[:], rl[:])
            osum = smallp.tile([n_walks, 1], F32)
            nc.vector.tensor_reduce(
                osum[:],
                sp[:],
                axis=mybir.AxisListType.X,
                op=mybir.AluOpType.add,
            )
            nc.sync.dma_start(out=out[:], in_=osum[:])

    nc.compile()
    return nc


_NC_CACHE = {}


def _get_nc(key):
    if key not in _NC_CACHE:
        _NC_CACHE[key] = build_kernel(*key)
    return _NC_CACHE[key]


def make_in_maps(walk, neg, embed, n_cores=N_CORES):
    B, L = walk.shape
    A, NEG = neg.shape[1], neg.shape[2]
    nw = B // n_cores
    embed_f = np.ascontiguousarray(embed.astype(np.float32, copy=False))
    in_maps = []
    for c in range(n_cores):
        sl = slice(c * nw, (c + 1) * nw)
        wslice = np.ascontiguousarray(walk[sl].astype(np.int32, copy=False))
        # neg [nw, A, NEG] -> plane-major [nw, NEG*A]
        nslice = np.ascontiguousarray(
            neg[sl].astype(np.int32, copy=False).transpose(0, 2, 1).reshape(nw, NEG * A)
        )
        in_maps.append({"walk_idx": wslice, "neg_idx": nslice, "embed": embed_f})
    return in_maps


def kernel(walk, neg, embed, _trace=False):
    walk = np.asarray(walk)
    neg = np.asarray(neg)
    embed = np.asarray(embed)
    B, L = walk.shape
    A, NEG = neg.shape[1], neg.shape[2]
    n_nodes, D = embed.shape

    nc = _get_nc((B // N_CORES, L, A, NEG, D, n_nodes, N_CORES))
    in_maps = make_in_maps(walk, neg, embed)
    res = run_bass_kernel_spmd(
        nc, in_maps, core_ids=list(range(N_CORES)), trace=_trace
    )
    total = 2 * B * A * NEG
    s = sum(r["out"].astype(np.float64).sum() for r in res.results)
    loss = np.float32(s / total)
    if _trace:
        return loss, res
    return loss
